# revision 55
# baseline (speedup 1.0000x reference)
# Trainium2 Bass kernel for nn_CauRecNet (2-layer residual-cell LSTM scan).
#
# v7 architecture (HW-calibrated):
#  - Pure data parallel across 8 cores; per core 8 "bodies" of 2 tiles
#    (1024 rows each), fully unrolled (no hardware loop, no per-body
#    all-engine barriers), with body n+1's prologue chunks interleaved
#    into body n's steady supersteps.
#  - Paired-tile supersteps: both tiles of a body advance together so each
#    tile's recurrence latency hides behind the other tile's work.
#  - Gates per cell computed as (i,f)/(o,g) pair-groups into [128,1024]
#    PSUM tiles from a 4-buffer pool: the PE runs up to two groups ahead
#    of the activations, keeping matmul streams dense (the tensor engine
#    p-state ramp makes gappy streams ~2-3x slower per matmul -- measured
#    175 ns/matmul continuous vs 645 ns ping-ponged on HW).
#  - One sigmoid per gate-pair (1024 wide); g-gate tanh via doubled
#    weights + 2*sig(2x)-1.  L0 runs three cells ahead; tanh of
#    [cres1b(t-1), cres0(t+2)] merged per tile and placed first in the
#    superstep (inputs land the previous superstep); cres1a(t) gets its
#    own tanh after the l1b sigmoids so the Act stream never head-of-line
#    blocks on a DVE chain.
#  - Elementwise cres chain stays entirely on DVE (fp16/bf16 operands ->
#    2x/4x modes); the c-state updates run on the idle GPSIMD/Pool engine
#    (off the critical path); fp16 cell states.
#  - x transposed on PE once per tile, repacked to matmul layout with
#    per-step SBUF DMAs; both tiles of a body share one xt allocation
#    (A at partition 0, B at 32, with a duplicate weight copy at 32).
#
# Measured (R-repeat slope, 8 cores): ~2.25 ms vs 3.78 ms harness
# baseline; rel err 5.2e-3.

import numpy as np
import ml_dtypes

B, T, F = 131072, 15, 12
H1, H2, CS = 64, 128, 96
NCORES = 8
BL = B // NCORES          # 16384 rows per core
NT = 512                  # matmul free dim (one half)
NPAIR = BL // (2 * NT)    # 16 pair-tiles per core
NBODY = NPAIR // 2        # 8 two-tile bodies

BF16 = ml_dtypes.bfloat16

_BUILD_CACHE = {}


def _build_bass(has_gate_bias, has_vec_bias, repeat=1):
    import os
    import concourse.bacc as bacc
    import concourse.tile as tile
    from concourse import mybir
    from concourse.masks import make_identity

    f32 = mybir.dt.float32
    bf16 = mybir.dt.bfloat16
    fp16 = mybir.dt.float16
    AF = mybir.ActivationFunctionType
    ALU = mybir.AluOpType

    nc = bacc.Bacc()

    # ---- DRAM I/O ----
    x_d = nc.dram_tensor("input_seq", [BL, T, F], f32, kind="ExternalInput")
    cs_d = nc.dram_tensor("cell_state", [BL, CS], f32, kind="ExternalInput")
    w0ih_d = nc.dram_tensor("w0ih_bd", [2 * F, 4 * H1 * 2], bf16, kind="ExternalInput")
    w0hh_d = nc.dram_tensor("w0hh_bd", [2 * H1, 4 * H1 * 2], bf16, kind="ExternalInput")
    w1ihA_d = nc.dram_tensor("w1ih_A", [2 * H1, 4 * H2], bf16, kind="ExternalInput")
    w1ihB_d = nc.dram_tensor("w1ih_B", [2 * H1, 4 * H2], bf16, kind="ExternalInput")
    w1hh_d = nc.dram_tensor("w1hhT", [H2, 4 * H2], bf16, kind="ExternalInput")
    fc1A_d = nc.dram_tensor("fc1_A", [CS, 2 * H1], f32, kind="ExternalInput")
    fc1B_d = nc.dram_tensor("fc1_B", [CS, 2 * H1], f32, kind="ExternalInput")
    fc2_d = nc.dram_tensor("fc2T", [CS, H2], f32, kind="ExternalInput")
    d1_d = nc.dram_tensor("d1T", [H2, H1], bf16, kind="ExternalInput")
    d2_d = nc.dram_tensor("d2T", [H1, 1], bf16, kind="ExternalInput")
    gb_d = nc.dram_tensor("gate_bias", [128, 8], f32, kind="ExternalInput")
    vb_d = nc.dram_tensor("vec_bias", [128, 4], f32, kind="ExternalInput")
    pred_d = nc.dram_tensor("pred", [BL, 1], f32, kind="ExternalOutput")

    # views indexed [body, u(tile within body), ...]
    x_view = x_d[:].rearrange("(n u c p) t f -> n u p c (t f)", u=2, c=8, p=128)
    cs_view = cs_d[:].rearrange("(n u c p) k -> n u p c k", u=2, c=8, p=128)
    pred_view = pred_d[:].rearrange("(n u h x) o -> n u h o x", u=2, h=2, x=NT)

    with tile.TileContext(nc) as tc:
        import contextlib
        ctx = contextlib.ExitStack()
        with ctx:
            consts = ctx.enter_context(tc.tile_pool(name="consts", bufs=1))
            loads = ctx.enter_context(tc.tile_pool(name="loads", bufs=2))
            xts = ctx.enter_context(tc.tile_pool(name="xts", bufs=2))
            states = ctx.enter_context(tc.tile_pool(name="states", bufs=3))
            states2 = ctx.enter_context(tc.tile_pool(name="states2", bufs=2))
            scratch = ctx.enter_context(tc.tile_pool(name="scratch", bufs=2))
            outp = ctx.enter_context(tc.tile_pool(name="outp", bufs=2))
            xtall = ctx.enter_context(tc.tile_pool(name="xtall", bufs=2))
            DWARM = int(os.environ.get("DWARM", "0"))
            GPAIR = int(os.environ.get("GPAIR", "0"))
            NPOOL = int(os.environ.get("NPOOL", "1"))   # 1, 2, or 4
            ABL = os.environ.get("ABL", "")   # "pe": matmuls only
            ppools = [
                ctx.enter_context(tc.tile_pool(
                    name=f"pp{i}", bufs=(2 if GPAIR else 4) // NPOOL,
                    space="PSUM"))
                for i in range(NPOOL)
            ]
            pp = ppools[0]
            _gctr = [0]

            def g_tile(name):
                # steady gate-group tile; rotate across pools so bursts of
                # (4 // NPOOL * NPOOL)... consecutive groups hit distinct
                # pools and need not wait the immediately preceding sig
                i = (_gctr[0] // (4 // NPOOL if NPOOL < 4 else 1)) % NPOOL \
                    if NPOOL > 1 else 0
                _gctr[0] += 1
                return ppools[i].tile([128, 1024], f32, tag="P", name=name)

            def pg_tile(name):
                # prologue/head scratch PSUM tile ([128,1024] usable region)
                if GPAIR:
                    t_ = pp.tile([128, 2048], f32, tag="P2", name=name)
                    return t_[:, 0:1024]
                return pp.tile([128, 1024], f32, tag="P", name=name)

            ident = consts.tile([128, 128], f32)
            make_identity(nc, ident)
            hconst = consts.tile([128, NT], mybir.dt.bfloat16, name="hconst")
            nc.vector.memset(hconst[:], 0.25)

            def load_const(name, dram, shape, dt):
                t = consts.tile(shape, dt, name=name)
                nc.sync.dma_start(out=t, in_=dram[:])
                return t

            w0ih_pair = consts.tile([32 + 2 * F, 512], bf16, name="w0ih_pair")
            nc.sync.dma_start(out=w0ih_pair[0:2 * F, :], in_=w0ih_d[:])
            nc.sync.dma_start(out=w0ih_pair[32:32 + 2 * F, :], in_=w0ih_d[:])
            w0hh = load_const("w0hh", w0hh_d, [2 * H1, 512], bf16)
            w1ihA = load_const("w1ihA", w1ihA_d, [2 * H1, 512], bf16)
            w1ihB = load_const("w1ihB", w1ihB_d, [2 * H1, 512], bf16)
            w1hh = load_const("w1hh", w1hh_d, [H2, 512], bf16)
            fc1A = load_const("fc1A", fc1A_d, [CS, 128], f32)
            fc1B = load_const("fc1B", fc1B_d, [CS, 128], f32)
            fc2 = load_const("fc2", fc2_d, [CS, H2], f32)
            d1w = load_const("d1w", d1_d, [H2, H1], bf16)
            d2w = load_const("d2w", d2_d, [H1, 1], bf16)
            gbias = load_const("gbias", gb_d, [128, 8], f32)
            vbias = load_const("vbias", vb_d, [128, 4], f32)


            def pe_warm(G, n):
                # n dummy 256-row matmuls into G's first region; the real
                # ih matmul (start=True) overwrites them
                for _ in range(n):
                    nc.tensor.matmul(G[:, 0:256], w1hh[:, 0:128],
                                     w1hh[:, 0:256], start=True, stop=True)

            # ---------- cell pieces (st carries one tile's live state) ----
            # gates are computed in (i,f)/(o,g) pair-groups of [128,1024] so
            # four PSUM buffers keep the PE two groups ahead of the
            # activations (sustained PE clock)
            def l0_group(st, t, og, G):
                x_t = st["xt_all"][:, t * NT:(t + 1) * NT]
                base = st["xt_base"]
                w0ih = w0ih_pair[base:base + 2 * F, :]
                if og == 0:
                    pe_warm(G, DWARM)
                for k in range(2):
                    gi = 2 * og + k
                    reg = G[:, k * 512:(k + 1) * 512]
                    nc.tensor.matmul(reg, w0ih[:, gi * 128:(gi + 1) * 128],
                                     x_t, start=True, stop=(t == 0))
                    if t > 0:
                        nc.tensor.matmul(reg, w0hh[:, gi * 128:(gi + 1) * 128],
                                         hconst if ABL else st["h0"][t - 1],
                                         start=False, stop=True)

            def l0_sigs(st, t, nm):
                if GPAIR:
                    Gp = pp.tile([128, 2048], f32, tag="P2", name="G0p")
                    l0_group(st, t, 0, Gp[:, 0:1024])
                    s_if = sig_of(Gp[:, 0:1024], "s0if", 1024)
                    l0_group(st, t, 1, Gp[:, 1024:2048])
                    s_og = sig_of(Gp[:, 1024:2048], nm, 1024)
                else:
                    Gif = g_tile("G0if")
                    l0_group(st, t, 0, Gif)
                    s_if = sig_of(Gif, "s0if", 1024)
                    Gog = g_tile("G0og")
                    l0_group(st, t, 1, Gog)
                    s_og = sig_of(Gog, nm, 1024)
                return s_if, s_og

            def l1_group(st, t, hf, og, G):
                w1ih = w1ihA if hf == 0 else w1ihB
                if hf == 1 and og == 0:
                    pe_warm(G, DWARM)
                for k in range(2):
                    ci = 2 * og + k
                    reg = G[:, k * 512:(k + 1) * 512]
                    nc.tensor.matmul(reg, w1ih[:, ci * 128:(ci + 1) * 128],
                                     hconst if ABL else st["h0"][t],
                                     start=True, stop=(t == 0))
                    if t > 0:
                        nc.tensor.matmul(reg, w1hh[:, ci * 128:(ci + 1) * 128],
                                         hconst if ABL else st["h1"][hf],
                                         start=False, stop=True)

            def l1_sigs(st, t, hf, nm):
                if GPAIR:
                    Gp = pp.tile([128, 2048], f32, tag="P2", name=f"G1p{hf}")
                    l1_group(st, t, hf, 0, Gp[:, 0:1024])
                    s_if = sig_of(Gp[:, 0:1024], "s1if", 1024)
                    l1_group(st, t, hf, 1, Gp[:, 1024:2048])
                    s_og = sig_of(Gp[:, 1024:2048], nm, 1024)
                else:
                    Gif = g_tile(f"G1{hf}if")
                    l1_group(st, t, hf, 0, Gif)
                    s_if = sig_of(Gif, "s1if", 1024)
                    Gog = g_tile(f"G1{hf}og")
                    l1_group(st, t, hf, 1, Gog)
                    s_og = sig_of(Gog, nm, 1024)
                return s_if, s_og

            def sig_of(G, nm, width=2048):
                s = scratch.tile([128, width], fp16, tag=f"sig{nm}",
                                 name=f"sig{nm}")
                if ABL != "pe":
                    nc.scalar.activation(s, G, AF.Sigmoid)
                if ABL == "pe":
                    # minimal consumer so the PSUM buffer is freed: read one
                    # column on DVE (cheap) instead of the sigmoid
                    nc.vector.tensor_copy(out=s[:, 0:8], in_=G[:, 0:8])
                return s

            def muls_of(s_if, s_og, c_in, nm, out_tile, out_slice):
                if ABL:
                    return
                # cres = sig_f*c + sig_i*(2*sig_g-1), written into
                # out_tile[:, out_slice] (fp16, SBUF -> DVE 2x/4x modes);
                # t1 runs on GPSIMD/Pool to shorten the DVE critical chain
                s_i = s_if[:, 0:512]
                s_f = s_if[:, 512:1024]
                s_g = s_og[:, 512:1024]
                g_t = scratch.tile([128, NT], fp16, tag=f"g{nm}", name=f"g{nm}")
                nc.vector.tensor_scalar(out=g_t, in0=s_g,
                                        scalar1=2.0, scalar2=1.0,
                                        op0=ALU.mult, op1=ALU.subtract)
                t2 = scratch.tile([128, NT], bf16, tag=f"t2{nm}", name=f"t2{nm}")
                nc.vector.tensor_mul(t2, s_i, g_t)
                t1 = scratch.tile([128, NT], bf16, tag=f"t1{nm}", name=f"t1{nm}")
                nc.vector.tensor_mul(t1, s_f, c_in)
                nc.vector.tensor_add(out_tile[:, out_slice], t1, t2)

            def cupd_of(c_in, cres, tag):
                if ABL:
                    return c_in
                # c updates run on the (otherwise idle) GPSIMD/Pool engine
                cn = states2.tile(c_in.shape, fp16, tag=tag, name=f"c_{tag}")
                nc.gpsimd.tensor_add(cn, c_in, cres)
                return cn

            def hmul(sig_o, th, tag, hshape):
                if ABL:
                    return None
                pool_ = states if tag.startswith("h0") and "w" not in tag \
                    else states2
                h = pool_.tile(hshape, bf16, tag=tag, name=f"h{tag}")
                nc.vector.tensor_mul(h, sig_o, th)
                return h

            # ---------- prologue, split into interleavable chunks ----------
            def warm_l0(st, t, full):
                u = st["sfx"]
                s_if, s_og = l0_sigs(st, t, f"0{u}")
                if full:
                    # sig+cres only; the tanh+hmul runs in the NEXT chunk so
                    # this chunk's Act stream never stalls on the DVE chain
                    crw = scratch.tile([128, NT], fp16, tag="crw",
                                       name="crw")
                    muls_of(s_if, s_og, st["c0"], "0", crw, slice(0, 512))
                    st["c0"] = cupd_of(st["c0"], crw, f"c0{u}")
                    st["warm_crw"], st["warm_og"], st["warm_t"] = \
                        crw, s_og, t
                else:
                    # cell whose tanh happens at steady t=0: write cres0
                    # into the body-shared pair tile at this tile's slot
                    shared = st["shared"]
                    if "crBC0" not in shared:
                        shared["crBC0"] = scratch.tile(
                            [128, 2048], fp16, tag="crBCp", name="crBCp")
                    ca = shared["crBC0"]
                    off = 512 if u == "0" else 1536
                    muls_of(s_if, s_og, st["c0"], "0", ca,
                            slice(off, off + 512))
                    st["c0"] = cupd_of(st["c0"], ca[:, off:off + 512],
                                       f"c0{u}")
                    st["crBC_pair"] = ca
                    st["sig0"] = s_og

            def prologue_chunks(body, u, shared):
                st = {"h0": {}, "h1": [None, None], "sfx": str(u),
                      "shared": shared}

                def c1_load_xpose():
                    x_nat = loads.tile([128, 8, T * F], f32, tag="x_nat")
                    nc.sync.dma_start(out=x_nat, in_=x_view[body][u])
                    cs_nat = loads.tile([128, 8, CS], f32, tag="cs_nat")
                    nc.sync.dma_start(out=cs_nat, in_=cs_view[body][u])
                    st["cs_nat"] = cs_nat
                    tp_x = pg_tile("tp_x")
                    for c in range(8):
                        nc.tensor.transpose(tp_x[0:96, c * 128:(c + 1) * 128],
                                            x_nat[:, c, 0:96], ident)
                    xT_lo = xts.tile([96, 1024], bf16, tag="xT_lo")
                    nc.vector.tensor_copy(out=xT_lo, in_=tp_x[0:96, 0:1024])
                    tp_x2 = pg_tile("tp_x2")
                    for c in range(8):
                        nc.tensor.transpose(
                            tp_x2[0:96, c * 128:(c + 1) * 128],
                            x_nat[:, c, 84:180], ident)
                    xT_hi = xts.tile([96, 1024], bf16, tag="xT_hi")
                    nc.vector.tensor_copy(out=xT_hi, in_=tp_x2[0:96, 0:1024])
                    st["xT"] = (xT_lo, xT_hi)

                def c2_cs_repack():
                    tp_c = pg_tile("tp_c")
                    for c in range(8):
                        nc.tensor.transpose(tp_c[0:96, c * 128:(c + 1) * 128],
                                            st["cs_nat"][:, c, :], ident)
                    csT = xts.tile([96, 1024], f32, tag="csT")
                    nc.vector.tensor_copy(out=csT, in_=tp_c[0:96, 0:1024])
                    st["csT"] = csT
                    # xt_all repack: per-t SBUF DMAs on the SP queue; only
                    # the t=0 slice gates the L0 warmup.  Both tiles of a
                    # body share one allocation (A at partition 0, B at 32)
                    if "xtpair" not in shared:
                        shared["xtpair"] = xtall.tile([32 + 2 * F, T * NT],
                                                      bf16, tag="xt_all",
                                                      name="xtpair")
                    base = 0 if u == 0 else 32
                    st["xt_base"] = base
                    xt_all = shared["xtpair"][base:base + 2 * F, :]
                    xT_lo, xT_hi = st["xT"]
                    for t in range(T):
                        src = (xT_lo[12 * t:12 * t + 12, :] if t < 8
                               else xT_hi[12 * t - 84:12 * t - 72, :])
                        nc.sync.dma_start(
                            out=xt_all[:, t * NT:(t + 1) * NT],
                            in_=src.rearrange("p (h x) -> p h x", h=2))
                    st["xt_all"] = xt_all

                def c3_init():
                    u_ = st["sfx"]
                    csT = st["csT"]
                    ip = pg_tile("ip")
                    nc.tensor.matmul(ip[:, 0:512], fc1A, csT[:, 0:512],
                                     start=True, stop=False)
                    nc.tensor.matmul(ip[:, 0:512], fc1B, csT[:, 512:1024],
                                     start=False, stop=True)
                    nc.tensor.matmul(ip[:, 512:1024], fc2, csT[:, 0:512],
                                     start=True, stop=True)
                    ip2 = pg_tile("ip2")
                    nc.tensor.matmul(ip2[:, 0:512], fc2, csT[:, 512:1024],
                                     start=True, stop=True)
                    c0 = states2.tile([128, NT], fp16, tag=f"c0i{u_}")
                    c1A = states2.tile([H2, NT], fp16, tag=f"c1Ai{u_}")
                    c1B = states2.tile([H2, NT], fp16, tag=f"c1Bi{u_}")
                    if has_vec_bias:
                        nc.vector.tensor_scalar_add(c0, ip[:, 0:512], vbias[:, 0:1])
                        nc.vector.tensor_scalar_add(c1A, ip[:, 512:1024],
                                                    vbias[:, 1:2])
                        nc.vector.tensor_scalar_add(c1B, ip2[:, 0:512],
                                                    vbias[:, 1:2])
                    else:
                        nc.vector.tensor_copy(out=c0, in_=ip[:, 0:512])
                        nc.vector.tensor_copy(out=c1A, in_=ip[:, 512:1024])
                        nc.vector.tensor_copy(out=c1B, in_=ip2[:, 0:512])
                    st["c0"], st["c1"] = c0, [c1A, c1B]

                def warm_fin():
                    if ABL:
                        return
                    u_ = st["sfx"]
                    crw, s_og, t_ = (st["warm_crw"], st["warm_og"],
                                     st["warm_t"])
                    thw = scratch.tile([128, NT], bf16, tag="thw",
                                       name="thw")
                    nc.scalar.activation(thw, crw, AF.Tanh)
                    st["h0"][t_] = hmul(s_og[:, 0:512], thw, f"h0w{u_}",
                                        [128, NT])

                def c4_l0_warm0():
                    warm_l0(st, 0, True)

                def c5_l0_warm1():
                    warm_fin()
                    warm_l0(st, 1, True)

                def c6_l0_warm2():
                    warm_fin()
                    warm_l0(st, 2, False)

                return st, [c1_load_xpose, c2_cs_repack, c3_init,
                            c4_l0_warm0, c5_l0_warm1, c6_l0_warm2]

            # ---------- one steady superstep for a pair of tiles ----
            # Emission order groups both tiles' matmuls into one long PE
            # run per superstep: G1A of both tiles first (deps ready at
            # step start), then G1B / L0 whose h-inputs emerge from the
            # early merged tanh while PE grinds G1A.
            def step_tanh(st, t):
                # per-tile merged tanh of [cres1b(t-1), cres0(t+2)] -- both
                # computed last superstep, read from this tile's half of the
                # shared pair tile
                if ABL:
                    return
                u = st["sfx"]
                boff = 0 if u == "0" else 1024
                crBC = st["crBC_pair"]
                lo = boff + (0 if t > 0 else 512)   # slot0 = cres1b(t-1)
                hi = boff + (1024 if t + 2 < T else 512)
                thBC = scratch.tile([128, hi - lo], bf16, tag="thBC",
                                    name="thBC")
                nc.scalar.activation(thBC, crBC[:, lo:hi], AF.Tanh)
                if t > 0:
                    st["h1"][1] = hmul(st["sig1b"][:, 0:512],
                                       thBC[:, 0:512], f"h1B{u}", [H2, NT])
                if t + 2 < T:
                    st["h0"][t + 2] = hmul(
                        st["sig0"][:, 0:512],
                        thBC[:, 512 - (lo - boff):1024 - (lo - boff)],
                        f"h0{u}", [128, NT])
                    st["h0"].pop(t - 2, None)

            def step_l1a(st, t):
                u = st["sfx"]
                aif, aog = l1_sigs(st, t, 0, "1a")
                cr1a = st["cr1a_pair"]
                off = 0 if u == "0" else 512
                muls_of(aif, aog, st["c1"][0], "1a", cr1a,
                        slice(off, off + 512))
                st["c1"][0] = cupd_of(st["c1"][0], cr1a[:, off:off + 512],
                                      f"c1A{u}")
                st["s1ao"] = aog

            def step_h1a2(stA, stB, t):
                if ABL:
                    return
                th1a = scratch.tile([128, 1024], bf16, tag="th1a",
                                    name="th1a")
                nc.scalar.activation(th1a, stA["cr1a_pair"], AF.Tanh)
                for st, off in ((stA, 0), (stB, 512)):
                    st["h1"][0] = hmul(st["s1ao"][:, 0:512],
                                       th1a[:, off:off + 512],
                                       f"h1A{st['sfx']}", [H2, NT])

            def step_l1b(st, t):
                u = st["sfx"]
                crBC_new = st["crBC_pair_new"]
                off = 0 if u == "0" else 1024
                bif, bog = l1_sigs(st, t, 1, f"1b{u}")
                muls_of(bif, bog, st["c1"][1], "1b", crBC_new,
                        slice(off, off + 512))
                st["c1"][1] = cupd_of(st["c1"][1],
                                      crBC_new[:, off:off + 512], f"c1B{u}")
                st["sig1b"] = bog

            def step_l0(st, t):
                u = st["sfx"]
                crBC_new = st["crBC_pair_new"]
                off = 512 if u == "0" else 1536
                if t + 3 < T:
                    s0if, s0og = l0_sigs(st, t + 3, f"0{u}")
                    muls_of(s0if, s0og, st["c0"], "0", crBC_new,
                            slice(off, off + 512))
                    st["c0"] = cupd_of(st["c0"], crBC_new[:, off:off + 512],
                                       f"c0{u}")
                    st["sig0"] = s0og

            def per_superstep(stA, stB, t):
                crBC_new = scratch.tile([128, 2048], fp16, tag="crBCp",
                                        name="crBCp")
                cr1a = scratch.tile([128, 1024], fp16, tag="cr1ap",
                                    name="cr1ap")
                for st in (stA, stB):
                    st["crBC_pair_new"] = crBC_new
                    st["cr1a_pair"] = cr1a
                step_tanh(stA, t)
                step_tanh(stB, t)
                step_l1a(stA, t)
                step_l1a(stB, t)
                step_l1b(stA, t)
                step_l1b(stB, t)
                step_h1a2(stA, stB, t)
                step_l0(stA, t)
                step_l0(stB, t)
                stA["crBC_pair"] = stB["crBC_pair"] = crBC_new

            def head(st, body, u):
                # final deferred L1b tanh, then the d1/d2 output head
                if not ABL:
                    thB = scratch.tile([128, NT], bf16, tag="thB",
                                       name="thB")
                    boff = 0 if st["sfx"] == "0" else 1024
                    nc.scalar.activation(
                        thB, st["crBC_pair"][:, boff:boff + 512], AF.Tanh)
                    st["h1"][1] = hmul(st["sig1b"][:, 0:512], thB,
                                       f"h1B{st['sfx']}", [H2, NT])
                h1 = st["h1"] if not ABL else [hconst, hconst]
                hp = pg_tile("hp")
                for hf in range(2):
                    nc.tensor.matmul(hp[0:H1, hf * 512:(hf + 1) * 512], d1w,
                                     h1[hf], start=True, stop=True)
                hp2 = pg_tile("hp2")
                for hf in range(2):
                    z = outp.tile([H1, NT], bf16, tag="z")
                    if has_vec_bias:
                        nc.vector.tensor_scalar_add(
                            z, hp[0:H1, hf * 512:(hf + 1) * 512],
                            vbias[0:H1, 2:3])
                    else:
                        nc.vector.tensor_copy(
                            out=z, in_=hp[0:H1, hf * 512:(hf + 1) * 512])
                    nc.tensor.matmul(
                        hp2[0:1, hf * 512:(hf + 1) * 512],
                        d2w, z, start=True, stop=True)
                    out_sb = outp.tile([1, NT], f32, tag="out_sb")
                    if has_vec_bias:
                        nc.vector.tensor_scalar_add(
                            out_sb,
                            hp2[0:1, hf * 512:(hf + 1) * 512],
                            vbias[0:1, 3:4])
                    else:
                        nc.vector.tensor_copy(
                            out=out_sb,
                            in_=hp2[0:1, hf * 512:(hf + 1) * 512])
                    nc.sync.dma_start(out=pred_view[body][u][hf], in_=out_sb)

            def paired_steady(stA, stB, body, interleave):
                # both tiles of one body advance together: each tile's
                # recurrence latency hides behind the other tile's work
                for t in range(T):
                    per_superstep(stA, stB, t)
                    for ch in interleave.get(t, []):
                        ch()
                head(stA, body, 0)
                head(stB, body, 1)

            def whole_workload(n_bodies):
                # body 0 prologue runs bare; bodies n+1 prologues interleave
                # into body n's steady supersteps
                shared0 = {}
                stA, chA = prologue_chunks(0, 0, shared0)
                stB, chB = prologue_chunks(0, 1, shared0)
                for ca_, cb_ in zip(chA, chB):
                    ca_()
                    cb_()
                for n in range(n_bodies):
                    inter = {}
                    if n + 1 < n_bodies:
                        shared2 = {}
                        stA2, chA2 = prologue_chunks(n + 1, 0, shared2)
                        stB2, chB2 = prologue_chunks(n + 1, 1, shared2)
                        inter = {2: [chA2[0]], 3: [chB2[0]],
                                 8: [chA2[1]], 9: [chB2[1]],
                                 10: [chA2[2]], 11: [chB2[2]],
                                 12: [chA2[3], chB2[3]],
                                 13: [chA2[4], chB2[4]],
                                 14: [chA2[5], chB2[5]]}
                    paired_steady(stA, stB, n, inter)
                    if n + 1 < n_bodies:
                        stA, stB = stA2, stB2

            n_unroll = int(os.environ.get("SIM_UNROLL", "0"))
            if n_unroll:
                whole_workload(n_unroll)
            elif repeat == 1:
                whole_workload(NBODY)
            else:  # benchmark variant: run the whole workload `repeat` times
                with tc.For_i(0, repeat, 1) as _r:
                    whole_workload(NBODY)

    nc.finalize()
    return nc


def _get_nc(key):
    if key not in _BUILD_CACHE:
        _BUILD_CACHE[key] = _build_bass(*key)
    return _BUILD_CACHE[key]


def _prep_weights(inputs):
    # gate order permutation i,f,g,o -> i,f,o,g (sigmoid gates contiguous)
    def perm(n):
        return np.concatenate([np.arange(0, 2 * n), np.arange(3 * n, 4 * n),
                               np.arange(2 * n, 3 * n)])
    p0, p1 = perm(H1), perm(H2)

    w0ihT = inputs["l0_w_ih"][p0].T.astype(np.float32)     # [12, 256]
    w0hhT = inputs["l0_w_hh"][p0].T.astype(np.float32)     # [64, 256]
    w1ihT = inputs["l1_w_ih"][p1].T.astype(np.float32)     # [64, 512]
    w1hhT = inputs["l1_w_hh"][p1].T.astype(np.float32)     # [128, 512]

    # g-gate (last quarter after perm) weights doubled: tanh(x) = 2*sig(2x)-1
    w0ihT[:, 3 * H1:] *= 2.0
    w0hhT[:, 3 * H1:] *= 2.0
    w1ihT[:, 3 * H2:] *= 2.0
    w1hhT[:, 3 * H2:] *= 2.0

    # L0 ih block-diagonal, rows interleaved (f,half) to match the repack DMA
    w0ih_bd = np.zeros((2 * F, 512), np.float32)
    w0ih_bd[0::2, :] = np.concatenate(
        [np.pad(w0ihT[:, g * 64:(g + 1) * 64], [(0, 0), (0, 64)])
         for g in range(4)], axis=1)                       # A rows -> cols 0:64 of each gate
    w0ih_bd[1::2, :] = np.concatenate(
        [np.pad(w0ihT[:, g * 64:(g + 1) * 64], [(0, 0), (64, 0)])
         for g in range(4)], axis=1)                       # B rows -> cols 64:128
    # L0 hh block-diagonal (A rows 0:64, B rows 64:128)
    w0hh_bd = np.zeros((2 * H1, 512), np.float32)
    for g in range(4):
        blk = w0hhT[:, g * 64:(g + 1) * 64]
        w0hh_bd[0:64, g * 128:g * 128 + 64] = blk
        w0hh_bd[64:128, g * 128 + 64:(g + 1) * 128] = blk
    # L1 ih half-masked (reads stacked h0)
    w1ih_A = np.concatenate([w1ihT, np.zeros_like(w1ihT)], axis=0)   # [128, 512]
    w1ih_B = np.concatenate([np.zeros_like(w1ihT), w1ihT], axis=0)
    fc1T = inputs["fc1_w"].T.astype(np.float32)            # [96, 64]
    fc1_A = np.concatenate([fc1T, np.zeros_like(fc1T)], axis=1)      # [96, 128]
    fc1_B = np.concatenate([np.zeros_like(fc1T), fc1T], axis=1)

    wm = {
        "w0ih_bd": w0ih_bd.astype(BF16),
        "w0hh_bd": w0hh_bd.astype(BF16),
        "w1ih_A": w1ih_A.astype(BF16),
        "w1ih_B": w1ih_B.astype(BF16),
        "w1hhT": np.ascontiguousarray(w1hhT).astype(BF16),
        "fc1_A": fc1_A,
        "fc1_B": fc1_B,
        "fc2T": np.ascontiguousarray(inputs["fc2_w"].T).astype(np.float32),
        "d1T": np.ascontiguousarray(inputs["d1_w"].T).astype(BF16),
        "d2T": np.ascontiguousarray(inputs["d2_w"].T).astype(BF16),
    }

    b0 = (inputs["l0_b_ih"] + inputs["l0_b_hh"]).astype(np.float32)[p0]   # [256]
    b1 = (inputs["l1_b_ih"] + inputs["l1_b_hh"]).astype(np.float32)[p1]   # [512]
    b0[3 * H1:] *= 2.0
    b1[3 * H2:] *= 2.0
    gb = np.zeros((128, 8), np.float32)
    for g in range(4):
        gb[:, g] = np.tile(b0[g * 64:(g + 1) * 64], 2)     # stacked [A;B]
        gb[:, 4 + g] = b1[g * 128:(g + 1) * 128]
    vb = np.zeros((128, 4), np.float32)
    vb[:, 0] = np.tile(inputs["fc1_b"], 2)
    vb[:, 1] = inputs["fc2_b"]
    vb[0:H1, 2] = inputs["d1_b"]
    vb[0:1, 3] = inputs["d2_b"]
    wm["gate_bias"] = gb
    wm["vec_bias"] = vb
    has_gate_bias = bool(np.any(b0) or np.any(b1))
    has_vec_bias = bool(np.any(vb))
    return wm, has_gate_bias, has_vec_bias


def _in_maps(inputs, wm):
    x = inputs["input_seq"].astype(np.float32, copy=False)
    cs = inputs["cell_state"].astype(np.float32, copy=False)
    maps = []
    for i in range(NCORES):
        m = dict(wm)
        m["input_seq"] = np.ascontiguousarray(x[i * BL:(i + 1) * BL])
        m["cell_state"] = np.ascontiguousarray(cs[i * BL:(i + 1) * BL])
        maps.append(m)
    return maps


def kernel(**inputs):
    inputs = {k: np.asarray(v) for k, v in inputs.items()}
    wm, hgb, hvb = _prep_weights(inputs)
    nc = _get_nc((hgb, hvb))
    from concourse.bass_utils import run_bass_kernel_spmd
    res = run_bass_kernel_spmd(nc, _in_maps(inputs, wm),
                               core_ids=list(range(NCORES)))
    return np.concatenate([r["pred"] for r in res.results], axis=0)


# revision 56
# speedup vs baseline: 1.0075x; 1.0075x over previous
# Trainium2 Bass kernel for nn_CauRecNet (2-layer residual-cell LSTM scan).
#
# v7 architecture (HW-calibrated):
#  - Pure data parallel across 8 cores; per core 8 "bodies" of 2 tiles
#    (1024 rows each), fully unrolled (no hardware loop, no per-body
#    all-engine barriers), with body n+1's prologue chunks interleaved
#    into body n's steady supersteps.
#  - Paired-tile supersteps: both tiles of a body advance together so each
#    tile's recurrence latency hides behind the other tile's work.
#  - Gates per cell computed as (i,f)/(o,g) pair-groups into [128,1024]
#    PSUM tiles from a 4-buffer pool: the PE runs up to two groups ahead
#    of the activations, keeping matmul streams dense (the tensor engine
#    p-state ramp makes gappy streams ~2-3x slower per matmul -- measured
#    175 ns/matmul continuous vs 645 ns ping-ponged on HW).
#  - One sigmoid per gate-pair (1024 wide); g-gate tanh via doubled
#    weights + 2*sig(2x)-1.  L0 runs three cells ahead; tanh of
#    [cres1b(t-1), cres0(t+2)] merged per tile and placed first in the
#    superstep (inputs land the previous superstep); both tiles'
#    cres1a(t) share one tanh placed after the l1b sigmoids so the Act
#    stream never head-of-line blocks on a DVE chain.
#  - Elementwise cres chain stays entirely on DVE (fp16/bf16 operands ->
#    2x/4x modes); the c-state updates run on the idle GPSIMD/Pool engine
#    (off the critical path); fp16 cell states.
#  - x transposed on PE once per tile, repacked to matmul layout with
#    per-step SBUF DMAs; both tiles of a body share one xt allocation
#    (A at partition 0, B at 32, with a duplicate weight copy at 32).
#
# Measured (R-repeat slope, 8 cores): ~2.25 ms vs 3.78 ms harness
# baseline; rel err 5.2e-3.

import numpy as np
import ml_dtypes

B, T, F = 131072, 15, 12
H1, H2, CS = 64, 128, 96
NCORES = 8
BL = B // NCORES          # 16384 rows per core
NT = 512                  # matmul free dim (one half)
NPAIR = BL // (2 * NT)    # 16 pair-tiles per core
NBODY = NPAIR // 2        # 8 two-tile bodies

BF16 = ml_dtypes.bfloat16

_BUILD_CACHE = {}


def _build_bass(has_gate_bias, has_vec_bias, repeat=1):
    import os
    import concourse.bacc as bacc
    import concourse.tile as tile
    from concourse import mybir
    from concourse.masks import make_identity

    f32 = mybir.dt.float32
    bf16 = mybir.dt.bfloat16
    fp16 = mybir.dt.float16
    AF = mybir.ActivationFunctionType
    ALU = mybir.AluOpType

    nc = bacc.Bacc()

    # ---- DRAM I/O ----
    x_d = nc.dram_tensor("input_seq", [BL, T, F], f32, kind="ExternalInput")
    cs_d = nc.dram_tensor("cell_state", [BL, CS], f32, kind="ExternalInput")
    w0ih_d = nc.dram_tensor("w0ih_bd", [2 * F, 4 * H1 * 2], bf16, kind="ExternalInput")
    w0hh_d = nc.dram_tensor("w0hh_bd", [2 * H1, 4 * H1 * 2], bf16, kind="ExternalInput")
    w1ihA_d = nc.dram_tensor("w1ih_A", [2 * H1, 4 * H2], bf16, kind="ExternalInput")
    w1ihB_d = nc.dram_tensor("w1ih_B", [2 * H1, 4 * H2], bf16, kind="ExternalInput")
    w1hh_d = nc.dram_tensor("w1hhT", [H2, 4 * H2], bf16, kind="ExternalInput")
    fc1A_d = nc.dram_tensor("fc1_A", [CS, 2 * H1], f32, kind="ExternalInput")
    fc1B_d = nc.dram_tensor("fc1_B", [CS, 2 * H1], f32, kind="ExternalInput")
    fc2_d = nc.dram_tensor("fc2T", [CS, H2], f32, kind="ExternalInput")
    d1_d = nc.dram_tensor("d1T", [H2, H1], bf16, kind="ExternalInput")
    d2_d = nc.dram_tensor("d2T", [H1, 1], bf16, kind="ExternalInput")
    gb_d = nc.dram_tensor("gate_bias", [128, 8], f32, kind="ExternalInput")
    vb_d = nc.dram_tensor("vec_bias", [128, 4], f32, kind="ExternalInput")
    pred_d = nc.dram_tensor("pred", [BL, 1], f32, kind="ExternalOutput")

    # views indexed [body, u(tile within body), ...]
    x_view = x_d[:].rearrange("(n u c p) t f -> n u p c (t f)", u=2, c=8, p=128)
    cs_view = cs_d[:].rearrange("(n u c p) k -> n u p c k", u=2, c=8, p=128)
    pred_view = pred_d[:].rearrange("(n u h x) o -> n u h o x", u=2, h=2, x=NT)

    with tile.TileContext(nc) as tc:
        import contextlib
        ctx = contextlib.ExitStack()
        with ctx:
            consts = ctx.enter_context(tc.tile_pool(name="consts", bufs=1))
            loads = ctx.enter_context(tc.tile_pool(name="loads", bufs=2))
            xts = ctx.enter_context(tc.tile_pool(name="xts", bufs=2))
            states = ctx.enter_context(tc.tile_pool(name="states", bufs=3))
            states2 = ctx.enter_context(tc.tile_pool(name="states2", bufs=2))
            scratch = ctx.enter_context(tc.tile_pool(name="scratch", bufs=2))
            outp = ctx.enter_context(tc.tile_pool(name="outp", bufs=2))
            xtall = ctx.enter_context(tc.tile_pool(name="xtall", bufs=2))
            DWARM = int(os.environ.get("DWARM", "0"))
            GPAIR = int(os.environ.get("GPAIR", "0"))
            NPOOL = int(os.environ.get("NPOOL", "1"))   # 1, 2, or 4
            ABL = os.environ.get("ABL", "")   # "pe": matmuls only
            ppools = [
                ctx.enter_context(tc.tile_pool(
                    name=f"pp{i}", bufs=(2 if GPAIR else 4) // NPOOL,
                    space="PSUM"))
                for i in range(NPOOL)
            ]
            pp = ppools[0]
            _gctr = [0]

            def g_tile(name):
                # steady gate-group tile; rotate across pools so bursts of
                # (4 // NPOOL * NPOOL)... consecutive groups hit distinct
                # pools and need not wait the immediately preceding sig
                i = (_gctr[0] // (4 // NPOOL if NPOOL < 4 else 1)) % NPOOL \
                    if NPOOL > 1 else 0
                _gctr[0] += 1
                return ppools[i].tile([128, 1024], f32, tag="P", name=name)

            def pg_tile(name):
                # prologue/head scratch PSUM tile ([128,1024] usable region)
                if GPAIR:
                    t_ = pp.tile([128, 2048], f32, tag="P2", name=name)
                    return t_[:, 0:1024]
                return pp.tile([128, 1024], f32, tag="P", name=name)

            ident = consts.tile([128, 128], f32)
            make_identity(nc, ident)
            hconst = consts.tile([128, NT], mybir.dt.bfloat16, name="hconst")
            nc.vector.memset(hconst[:], 0.25)

            def load_const(name, dram, shape, dt):
                t = consts.tile(shape, dt, name=name)
                nc.sync.dma_start(out=t, in_=dram[:])
                return t

            w0ih_pair = consts.tile([32 + 2 * F, 512], bf16, name="w0ih_pair")
            nc.sync.dma_start(out=w0ih_pair[0:2 * F, :], in_=w0ih_d[:])
            nc.sync.dma_start(out=w0ih_pair[32:32 + 2 * F, :], in_=w0ih_d[:])
            w0hh = load_const("w0hh", w0hh_d, [2 * H1, 512], bf16)
            w1ihA = load_const("w1ihA", w1ihA_d, [2 * H1, 512], bf16)
            w1ihB = load_const("w1ihB", w1ihB_d, [2 * H1, 512], bf16)
            w1hh = load_const("w1hh", w1hh_d, [H2, 512], bf16)
            fc1A = load_const("fc1A", fc1A_d, [CS, 128], f32)
            fc1B = load_const("fc1B", fc1B_d, [CS, 128], f32)
            fc2 = load_const("fc2", fc2_d, [CS, H2], f32)
            d1w = load_const("d1w", d1_d, [H2, H1], bf16)
            d2w = load_const("d2w", d2_d, [H1, 1], bf16)
            gbias = load_const("gbias", gb_d, [128, 8], f32)
            vbias = load_const("vbias", vb_d, [128, 4], f32)


            def pe_warm(G, n):
                # n dummy 256-row matmuls into G's first region; the real
                # ih matmul (start=True) overwrites them
                for _ in range(n):
                    nc.tensor.matmul(G[:, 0:256], w1hh[:, 0:128],
                                     w1hh[:, 0:256], start=True, stop=True)

            # ---------- cell pieces (st carries one tile's live state) ----
            # gates are computed in (i,f)/(o,g) pair-groups of [128,1024] so
            # four PSUM buffers keep the PE two groups ahead of the
            # activations (sustained PE clock)
            def l0_group(st, t, og, G):
                x_t = st["xt_all"][:, t * NT:(t + 1) * NT]
                base = st["xt_base"]
                w0ih = w0ih_pair[base:base + 2 * F, :]
                if og == 0:
                    pe_warm(G, DWARM)
                for k in range(2):
                    gi = 2 * og + k
                    reg = G[:, k * 512:(k + 1) * 512]
                    nc.tensor.matmul(reg, w0ih[:, gi * 128:(gi + 1) * 128],
                                     x_t, start=True, stop=(t == 0))
                    if t > 0:
                        nc.tensor.matmul(reg, w0hh[:, gi * 128:(gi + 1) * 128],
                                         hconst if ABL else st["h0"][t - 1],
                                         start=False, stop=True)

            def l0_sigs(st, t, nm):
                if GPAIR:
                    Gp = pp.tile([128, 2048], f32, tag="P2", name="G0p")
                    l0_group(st, t, 0, Gp[:, 0:1024])
                    s_if = sig_of(Gp[:, 0:1024], "s0if", 1024)
                    l0_group(st, t, 1, Gp[:, 1024:2048])
                    s_og = sig_of(Gp[:, 1024:2048], nm, 1024)
                else:
                    Gif = g_tile("G0if")
                    l0_group(st, t, 0, Gif)
                    s_if = sig_of(Gif, "s0if", 1024)
                    Gog = g_tile("G0og")
                    l0_group(st, t, 1, Gog)
                    s_og = sig_of(Gog, nm, 1024)
                return s_if, s_og

            def l1_group(st, t, hf, og, G):
                w1ih = w1ihA if hf == 0 else w1ihB
                if hf == 1 and og == 0:
                    pe_warm(G, DWARM)
                for k in range(2):
                    ci = 2 * og + k
                    reg = G[:, k * 512:(k + 1) * 512]
                    nc.tensor.matmul(reg, w1ih[:, ci * 128:(ci + 1) * 128],
                                     hconst if ABL else st["h0"][t],
                                     start=True, stop=(t == 0))
                    if t > 0:
                        nc.tensor.matmul(reg, w1hh[:, ci * 128:(ci + 1) * 128],
                                         hconst if ABL else st["h1"][hf],
                                         start=False, stop=True)

            def l1_sigs(st, t, hf, nm):
                if GPAIR:
                    Gp = pp.tile([128, 2048], f32, tag="P2", name=f"G1p{hf}")
                    l1_group(st, t, hf, 0, Gp[:, 0:1024])
                    s_if = sig_of(Gp[:, 0:1024], "s1if", 1024)
                    l1_group(st, t, hf, 1, Gp[:, 1024:2048])
                    s_og = sig_of(Gp[:, 1024:2048], nm, 1024)
                else:
                    Gif = g_tile(f"G1{hf}if")
                    l1_group(st, t, hf, 0, Gif)
                    s_if = sig_of(Gif, "s1if", 1024)
                    Gog = g_tile(f"G1{hf}og")
                    l1_group(st, t, hf, 1, Gog)
                    s_og = sig_of(Gog, nm, 1024)
                return s_if, s_og

            def sig_of(G, nm, width=2048):
                s = scratch.tile([128, width], fp16, tag=f"sig{nm}",
                                 name=f"sig{nm}")
                if ABL != "pe":
                    nc.scalar.activation(s, G, AF.Sigmoid)
                if ABL == "pe":
                    # minimal consumer so the PSUM buffer is freed: read one
                    # column on DVE (cheap) instead of the sigmoid
                    nc.vector.tensor_copy(out=s[:, 0:8], in_=G[:, 0:8])
                return s

            def muls_of(s_if, s_og, c_in, nm, out_tile, out_slice):
                if ABL:
                    return
                # cres = sig_f*c + sig_i*(2*sig_g-1), written into
                # out_tile[:, out_slice] (fp16, SBUF -> DVE 2x/4x modes);
                # t1 runs on GPSIMD/Pool to shorten the DVE critical chain
                s_i = s_if[:, 0:512]
                s_f = s_if[:, 512:1024]
                s_g = s_og[:, 512:1024]
                g_t = scratch.tile([128, NT], fp16, tag=f"g{nm}", name=f"g{nm}")
                nc.vector.tensor_scalar(out=g_t, in0=s_g,
                                        scalar1=2.0, scalar2=1.0,
                                        op0=ALU.mult, op1=ALU.subtract)
                t2 = scratch.tile([128, NT], bf16, tag=f"t2{nm}", name=f"t2{nm}")
                nc.vector.tensor_mul(t2, s_i, g_t)
                t1 = scratch.tile([128, NT], bf16, tag=f"t1{nm}", name=f"t1{nm}")
                nc.vector.tensor_mul(t1, s_f, c_in)
                nc.vector.tensor_add(out_tile[:, out_slice], t1, t2)

            def cupd_of(c_in, cres, tag):
                if ABL:
                    return c_in
                # c updates run on the (otherwise idle) GPSIMD/Pool engine
                cn = states2.tile(c_in.shape, fp16, tag=tag, name=f"c_{tag}")
                nc.gpsimd.tensor_add(cn, c_in, cres)
                return cn

            def hmul(sig_o, th, tag, hshape):
                if ABL:
                    return None
                pool_ = states if tag.startswith("h0") and "w" not in tag \
                    else states2
                h = pool_.tile(hshape, bf16, tag=tag, name=f"h{tag}")
                nc.vector.tensor_mul(h, sig_o, th)
                return h

            # ---------- prologue, split into interleavable chunks ----------
            def warm_l0(st, t, full):
                u = st["sfx"]
                s_if, s_og = l0_sigs(st, t, f"0{u}")
                if full:
                    # sig+cres only; the tanh+hmul runs in the NEXT chunk so
                    # this chunk's Act stream never stalls on the DVE chain
                    crw = scratch.tile([128, NT], fp16, tag="crw",
                                       name="crw")
                    muls_of(s_if, s_og, st["c0"], "0", crw, slice(0, 512))
                    st["c0"] = cupd_of(st["c0"], crw, f"c0{u}")
                    st["warm_crw"], st["warm_og"], st["warm_t"] = \
                        crw, s_og, t
                else:
                    # cell whose tanh happens at steady t=0: write cres0
                    # into the body-shared pair tile at this tile's slot
                    shared = st["shared"]
                    if "crBC0" not in shared:
                        shared["crBC0"] = scratch.tile(
                            [128, 2048], fp16, tag="crBCp", name="crBCp")
                    ca = shared["crBC0"]
                    off = 512 if u == "0" else 1536
                    muls_of(s_if, s_og, st["c0"], "0", ca,
                            slice(off, off + 512))
                    st["c0"] = cupd_of(st["c0"], ca[:, off:off + 512],
                                       f"c0{u}")
                    st["crBC_pair"] = ca
                    st["sig0"] = s_og

            def prologue_chunks(body, u, shared):
                st = {"h0": {}, "h1": [None, None], "sfx": str(u),
                      "shared": shared}

                def c1_load_xpose():
                    x_nat = loads.tile([128, 8, T * F], f32, tag="x_nat")
                    nc.sync.dma_start(out=x_nat, in_=x_view[body][u])
                    cs_nat = loads.tile([128, 8, CS], f32, tag="cs_nat")
                    nc.sync.dma_start(out=cs_nat, in_=cs_view[body][u])
                    st["cs_nat"] = cs_nat
                    tp_x = pg_tile("tp_x")
                    for c in range(8):
                        nc.tensor.transpose(tp_x[0:96, c * 128:(c + 1) * 128],
                                            x_nat[:, c, 0:96], ident)
                    xT_lo = xts.tile([96, 1024], bf16, tag="xT_lo")
                    nc.vector.tensor_copy(out=xT_lo, in_=tp_x[0:96, 0:1024])
                    tp_x2 = pg_tile("tp_x2")
                    for c in range(8):
                        nc.tensor.transpose(
                            tp_x2[0:96, c * 128:(c + 1) * 128],
                            x_nat[:, c, 84:180], ident)
                    xT_hi = xts.tile([96, 1024], bf16, tag="xT_hi")
                    nc.vector.tensor_copy(out=xT_hi, in_=tp_x2[0:96, 0:1024])
                    st["xT"] = (xT_lo, xT_hi)

                def c2_cs_repack():
                    tp_c = pg_tile("tp_c")
                    for c in range(8):
                        nc.tensor.transpose(tp_c[0:96, c * 128:(c + 1) * 128],
                                            st["cs_nat"][:, c, :], ident)
                    csT = xts.tile([96, 1024], f32, tag="csT")
                    nc.vector.tensor_copy(out=csT, in_=tp_c[0:96, 0:1024])
                    st["csT"] = csT
                    # xt_all repack: per-t SBUF DMAs on the SP queue; only
                    # the t=0 slice gates the L0 warmup.  Both tiles of a
                    # body share one allocation (A at partition 0, B at 32)
                    if "xtpair" not in shared:
                        shared["xtpair"] = xtall.tile([32 + 2 * F, T * NT],
                                                      bf16, tag="xt_all",
                                                      name="xtpair")
                    base = 0 if u == 0 else 32
                    st["xt_base"] = base
                    xt_all = shared["xtpair"][base:base + 2 * F, :]
                    xT_lo, xT_hi = st["xT"]
                    for t in range(T):
                        src = (xT_lo[12 * t:12 * t + 12, :] if t < 8
                               else xT_hi[12 * t - 84:12 * t - 72, :])
                        nc.sync.dma_start(
                            out=xt_all[:, t * NT:(t + 1) * NT],
                            in_=src.rearrange("p (h x) -> p h x", h=2))
                    st["xt_all"] = xt_all

                def c3_init():
                    u_ = st["sfx"]
                    csT = st["csT"]
                    ip = pg_tile("ip")
                    nc.tensor.matmul(ip[:, 0:512], fc1A, csT[:, 0:512],
                                     start=True, stop=False)
                    nc.tensor.matmul(ip[:, 0:512], fc1B, csT[:, 512:1024],
                                     start=False, stop=True)
                    nc.tensor.matmul(ip[:, 512:1024], fc2, csT[:, 0:512],
                                     start=True, stop=True)
                    ip2 = pg_tile("ip2")
                    nc.tensor.matmul(ip2[:, 0:512], fc2, csT[:, 512:1024],
                                     start=True, stop=True)
                    c0 = states2.tile([128, NT], fp16, tag=f"c0i{u_}")
                    c1A = states2.tile([H2, NT], fp16, tag=f"c1Ai{u_}")
                    c1B = states2.tile([H2, NT], fp16, tag=f"c1Bi{u_}")
                    if has_vec_bias:
                        nc.vector.tensor_scalar_add(c0, ip[:, 0:512], vbias[:, 0:1])
                        nc.vector.tensor_scalar_add(c1A, ip[:, 512:1024],
                                                    vbias[:, 1:2])
                        nc.vector.tensor_scalar_add(c1B, ip2[:, 0:512],
                                                    vbias[:, 1:2])
                    else:
                        nc.vector.tensor_copy(out=c0, in_=ip[:, 0:512])
                        nc.vector.tensor_copy(out=c1A, in_=ip[:, 512:1024])
                        nc.vector.tensor_copy(out=c1B, in_=ip2[:, 0:512])
                    st["c0"], st["c1"] = c0, [c1A, c1B]

                def warm_fin():
                    if ABL:
                        return
                    u_ = st["sfx"]
                    crw, s_og, t_ = (st["warm_crw"], st["warm_og"],
                                     st["warm_t"])
                    thw = scratch.tile([128, NT], bf16, tag="thw",
                                       name="thw")
                    nc.scalar.activation(thw, crw, AF.Tanh)
                    st["h0"][t_] = hmul(s_og[:, 0:512], thw, f"h0w{u_}",
                                        [128, NT])

                def c4_l0_warm0():
                    warm_l0(st, 0, True)

                def c5_l0_warm1():
                    warm_fin()
                    warm_l0(st, 1, True)

                def c6_l0_warm2():
                    warm_fin()
                    warm_l0(st, 2, False)

                return st, [c1_load_xpose, c2_cs_repack, c3_init,
                            c4_l0_warm0, c5_l0_warm1, c6_l0_warm2]

            # ---------- one steady superstep for a pair of tiles ----
            # Emission order groups both tiles' matmuls into one long PE
            # run per superstep: G1A of both tiles first (deps ready at
            # step start), then G1B / L0 whose h-inputs emerge from the
            # early merged tanh while PE grinds G1A.
            def step_tanh(st, t):
                # per-tile merged tanh of [cres1b(t-1), cres0(t+2)] -- both
                # computed last superstep, read from this tile's half of the
                # shared pair tile
                if ABL:
                    return
                u = st["sfx"]
                boff = 0 if u == "0" else 1024
                crBC = st["crBC_pair"]
                lo = boff + (0 if t > 0 else 512)   # slot0 = cres1b(t-1)
                hi = boff + (1024 if t + 2 < T else 512)
                thBC = scratch.tile([128, hi - lo], bf16, tag="thBC",
                                    name="thBC")
                nc.scalar.activation(thBC, crBC[:, lo:hi], AF.Tanh)
                if t > 0:
                    st["h1"][1] = hmul(st["sig1b"][:, 0:512],
                                       thBC[:, 0:512], f"h1B{u}", [H2, NT])
                if t + 2 < T:
                    st["h0"][t + 2] = hmul(
                        st["sig0"][:, 0:512],
                        thBC[:, 512 - (lo - boff):1024 - (lo - boff)],
                        f"h0{u}", [128, NT])
                    st["h0"].pop(t - 2, None)

            def step_l1a(st, t):
                u = st["sfx"]
                aif, aog = l1_sigs(st, t, 0, "1a")
                cr1a = st["cr1a_pair"]
                off = 0 if u == "0" else 512
                muls_of(aif, aog, st["c1"][0], "1a", cr1a,
                        slice(off, off + 512))
                st["c1"][0] = cupd_of(st["c1"][0], cr1a[:, off:off + 512],
                                      f"c1A{u}")
                st["s1ao"] = aog

            def step_h1a2(stA, stB, t):
                if ABL:
                    return
                th1a = scratch.tile([128, 1024], bf16, tag="th1a",
                                    name="th1a")
                nc.scalar.activation(th1a, stA["cr1a_pair"], AF.Tanh)
                for st, off in ((stA, 0), (stB, 512)):
                    st["h1"][0] = hmul(st["s1ao"][:, 0:512],
                                       th1a[:, off:off + 512],
                                       f"h1A{st['sfx']}", [H2, NT])

            def step_l1b(st, t):
                u = st["sfx"]
                crBC_new = st["crBC_pair_new"]
                off = 0 if u == "0" else 1024
                bif, bog = l1_sigs(st, t, 1, f"1b{u}")
                muls_of(bif, bog, st["c1"][1], "1b", crBC_new,
                        slice(off, off + 512))
                st["c1"][1] = cupd_of(st["c1"][1],
                                      crBC_new[:, off:off + 512], f"c1B{u}")
                st["sig1b"] = bog

            def step_l0(st, t):
                u = st["sfx"]
                crBC_new = st["crBC_pair_new"]
                off = 512 if u == "0" else 1536
                if t + 3 < T:
                    s0if, s0og = l0_sigs(st, t + 3, f"0{u}")
                    muls_of(s0if, s0og, st["c0"], "0", crBC_new,
                            slice(off, off + 512))
                    st["c0"] = cupd_of(st["c0"], crBC_new[:, off:off + 512],
                                       f"c0{u}")
                    st["sig0"] = s0og

            def per_superstep(stA, stB, t):
                crBC_new = scratch.tile([128, 2048], fp16, tag="crBCp",
                                        name="crBCp")
                cr1a = scratch.tile([128, 1024], fp16, tag="cr1ap",
                                    name="cr1ap")
                for st in (stA, stB):
                    st["crBC_pair_new"] = crBC_new
                    st["cr1a_pair"] = cr1a
                step_tanh(stA, t)
                step_tanh(stB, t)
                step_l1a(stA, t)
                step_l1a(stB, t)
                step_l1b(stA, t)
                step_l1b(stB, t)
                step_h1a2(stA, stB, t)
                step_l0(stA, t)
                step_l0(stB, t)
                stA["crBC_pair"] = stB["crBC_pair"] = crBC_new

            def head(st, body, u):
                # final deferred L1b tanh, then the d1/d2 output head
                if not ABL:
                    thB = scratch.tile([128, NT], bf16, tag="thB",
                                       name="thB")
                    boff = 0 if st["sfx"] == "0" else 1024
                    nc.scalar.activation(
                        thB, st["crBC_pair"][:, boff:boff + 512], AF.Tanh)
                    st["h1"][1] = hmul(st["sig1b"][:, 0:512], thB,
                                       f"h1B{st['sfx']}", [H2, NT])
                h1 = st["h1"] if not ABL else [hconst, hconst]
                hp = pg_tile("hp")
                for hf in range(2):
                    nc.tensor.matmul(hp[0:H1, hf * 512:(hf + 1) * 512], d1w,
                                     h1[hf], start=True, stop=True)
                hp2 = pg_tile("hp2")
                for hf in range(2):
                    z = outp.tile([H1, NT], bf16, tag="z")
                    if has_vec_bias:
                        nc.vector.tensor_scalar_add(
                            z, hp[0:H1, hf * 512:(hf + 1) * 512],
                            vbias[0:H1, 2:3])
                    else:
                        nc.vector.tensor_copy(
                            out=z, in_=hp[0:H1, hf * 512:(hf + 1) * 512])
                    nc.tensor.matmul(
                        hp2[0:1, hf * 512:(hf + 1) * 512],
                        d2w, z, start=True, stop=True)
                    out_sb = outp.tile([1, NT], f32, tag="out_sb")
                    if has_vec_bias:
                        nc.vector.tensor_scalar_add(
                            out_sb,
                            hp2[0:1, hf * 512:(hf + 1) * 512],
                            vbias[0:1, 3:4])
                    else:
                        nc.vector.tensor_copy(
                            out=out_sb,
                            in_=hp2[0:1, hf * 512:(hf + 1) * 512])
                    nc.sync.dma_start(out=pred_view[body][u][hf], in_=out_sb)

            def paired_steady(stA, stB, body, interleave):
                # both tiles of one body advance together: each tile's
                # recurrence latency hides behind the other tile's work
                for t in range(T):
                    per_superstep(stA, stB, t)
                    for ch in interleave.get(t, []):
                        ch()
                head(stA, body, 0)
                head(stB, body, 1)

            def whole_workload(n_bodies):
                # body 0 prologue runs bare; bodies n+1 prologues interleave
                # into body n's steady supersteps
                shared0 = {}
                stA, chA = prologue_chunks(0, 0, shared0)
                stB, chB = prologue_chunks(0, 1, shared0)
                for ca_, cb_ in zip(chA, chB):
                    ca_()
                    cb_()
                for n in range(n_bodies):
                    inter = {}
                    if n + 1 < n_bodies:
                        shared2 = {}
                        stA2, chA2 = prologue_chunks(n + 1, 0, shared2)
                        stB2, chB2 = prologue_chunks(n + 1, 1, shared2)
                        inter = {2: [chA2[0]], 3: [chB2[0]],
                                 8: [chA2[1]], 9: [chB2[1]],
                                 10: [chA2[2]], 11: [chB2[2]],
                                 12: [chA2[3], chB2[3]],
                                 13: [chA2[4], chB2[4]],
                                 14: [chA2[5], chB2[5]]}
                    paired_steady(stA, stB, n, inter)
                    if n + 1 < n_bodies:
                        stA, stB = stA2, stB2

            n_unroll = int(os.environ.get("SIM_UNROLL", "0"))
            if n_unroll:
                whole_workload(n_unroll)
            elif repeat == 1:
                whole_workload(NBODY)
            else:  # benchmark variant: run the whole workload `repeat` times
                with tc.For_i(0, repeat, 1) as _r:
                    whole_workload(NBODY)

    nc.finalize()
    return nc


def _get_nc(key):
    if key not in _BUILD_CACHE:
        _BUILD_CACHE[key] = _build_bass(*key)
    return _BUILD_CACHE[key]


def _prep_weights(inputs):
    # gate order permutation i,f,g,o -> i,f,o,g (sigmoid gates contiguous)
    def perm(n):
        return np.concatenate([np.arange(0, 2 * n), np.arange(3 * n, 4 * n),
                               np.arange(2 * n, 3 * n)])
    p0, p1 = perm(H1), perm(H2)

    w0ihT = inputs["l0_w_ih"][p0].T.astype(np.float32)     # [12, 256]
    w0hhT = inputs["l0_w_hh"][p0].T.astype(np.float32)     # [64, 256]
    w1ihT = inputs["l1_w_ih"][p1].T.astype(np.float32)     # [64, 512]
    w1hhT = inputs["l1_w_hh"][p1].T.astype(np.float32)     # [128, 512]

    # g-gate (last quarter after perm) weights doubled: tanh(x) = 2*sig(2x)-1
    w0ihT[:, 3 * H1:] *= 2.0
    w0hhT[:, 3 * H1:] *= 2.0
    w1ihT[:, 3 * H2:] *= 2.0
    w1hhT[:, 3 * H2:] *= 2.0

    # L0 ih block-diagonal, rows interleaved (f,half) to match the repack DMA
    w0ih_bd = np.zeros((2 * F, 512), np.float32)
    w0ih_bd[0::2, :] = np.concatenate(
        [np.pad(w0ihT[:, g * 64:(g + 1) * 64], [(0, 0), (0, 64)])
         for g in range(4)], axis=1)                       # A rows -> cols 0:64 of each gate
    w0ih_bd[1::2, :] = np.concatenate(
        [np.pad(w0ihT[:, g * 64:(g + 1) * 64], [(0, 0), (64, 0)])
         for g in range(4)], axis=1)                       # B rows -> cols 64:128
    # L0 hh block-diagonal (A rows 0:64, B rows 64:128)
    w0hh_bd = np.zeros((2 * H1, 512), np.float32)
    for g in range(4):
        blk = w0hhT[:, g * 64:(g + 1) * 64]
        w0hh_bd[0:64, g * 128:g * 128 + 64] = blk
        w0hh_bd[64:128, g * 128 + 64:(g + 1) * 128] = blk
    # L1 ih half-masked (reads stacked h0)
    w1ih_A = np.concatenate([w1ihT, np.zeros_like(w1ihT)], axis=0)   # [128, 512]
    w1ih_B = np.concatenate([np.zeros_like(w1ihT), w1ihT], axis=0)
    fc1T = inputs["fc1_w"].T.astype(np.float32)            # [96, 64]
    fc1_A = np.concatenate([fc1T, np.zeros_like(fc1T)], axis=1)      # [96, 128]
    fc1_B = np.concatenate([np.zeros_like(fc1T), fc1T], axis=1)

    wm = {
        "w0ih_bd": w0ih_bd.astype(BF16),
        "w0hh_bd": w0hh_bd.astype(BF16),
        "w1ih_A": w1ih_A.astype(BF16),
        "w1ih_B": w1ih_B.astype(BF16),
        "w1hhT": np.ascontiguousarray(w1hhT).astype(BF16),
        "fc1_A": fc1_A,
        "fc1_B": fc1_B,
        "fc2T": np.ascontiguousarray(inputs["fc2_w"].T).astype(np.float32),
        "d1T": np.ascontiguousarray(inputs["d1_w"].T).astype(BF16),
        "d2T": np.ascontiguousarray(inputs["d2_w"].T).astype(BF16),
    }

    b0 = (inputs["l0_b_ih"] + inputs["l0_b_hh"]).astype(np.float32)[p0]   # [256]
    b1 = (inputs["l1_b_ih"] + inputs["l1_b_hh"]).astype(np.float32)[p1]   # [512]
    b0[3 * H1:] *= 2.0
    b1[3 * H2:] *= 2.0
    gb = np.zeros((128, 8), np.float32)
    for g in range(4):
        gb[:, g] = np.tile(b0[g * 64:(g + 1) * 64], 2)     # stacked [A;B]
        gb[:, 4 + g] = b1[g * 128:(g + 1) * 128]
    vb = np.zeros((128, 4), np.float32)
    vb[:, 0] = np.tile(inputs["fc1_b"], 2)
    vb[:, 1] = inputs["fc2_b"]
    vb[0:H1, 2] = inputs["d1_b"]
    vb[0:1, 3] = inputs["d2_b"]
    wm["gate_bias"] = gb
    wm["vec_bias"] = vb
    has_gate_bias = bool(np.any(b0) or np.any(b1))
    has_vec_bias = bool(np.any(vb))
    return wm, has_gate_bias, has_vec_bias


def _in_maps(inputs, wm):
    x = inputs["input_seq"].astype(np.float32, copy=False)
    cs = inputs["cell_state"].astype(np.float32, copy=False)
    maps = []
    for i in range(NCORES):
        m = dict(wm)
        m["input_seq"] = np.ascontiguousarray(x[i * BL:(i + 1) * BL])
        m["cell_state"] = np.ascontiguousarray(cs[i * BL:(i + 1) * BL])
        maps.append(m)
    return maps


def kernel(**inputs):
    inputs = {k: np.asarray(v) for k, v in inputs.items()}
    wm, hgb, hvb = _prep_weights(inputs)
    nc = _get_nc((hgb, hvb))
    from concourse.bass_utils import run_bass_kernel_spmd
    res = run_bass_kernel_spmd(nc, _in_maps(inputs, wm),
                               core_ids=list(range(NCORES)))
    return np.concatenate([r["pred"] for r in res.results], axis=0)
